# revision 1
# baseline (speedup 1.0000x reference)
"""Trainium2 Bass kernel for nn_MultiHeadDaubechiesBlock.

Data-parallel over batch B=8 across 8 NeuronCores (one sequence per core).
Per-core pipeline (all bf16 GEMMs on PE, N=512):
  LN1 (DVE bn_stats) -> DWT as Toeplitz-block matmuls (token-major chain)
  -> linear-interp upsample as banded matmuls (feature-major out)
  -> proj GEMM + residual (rank-1 bias folds) -> x2 (DRAM staging)
  -> LN2 -> FFN1 + exact gelu (ACT LUT) -> FFN2 + residual -> out.
LN gains/biases are folded into the adjacent GEMV weights on the host
(exact for any g/b). Depthwise wavelet filters are assumed constant
across heads/channels (true for this module's inputs); filter values are
taken from the h0/h1 input tensors at runtime.
"""
import numpy as np
import ml_dtypes

B, T, D, H, DH, LEVELS, FFN = 8, 4096, 512, 4, 128, 3, 2048
P = 128
NT = T // P          # 32 token tiles
NDT = D // P         # 4 feature tiles
NFT = FFN // P       # 16 ffn tiles
NCH = 8              # t-chunks of 512
NWS = [2047, 1023, 511]
LPADS = [4096, 2048, 1024]
EPS = 1e-5
BF16 = ml_dtypes.bfloat16


# ----------------------------------------------------------------- plan
def _interp_mat(L, out_size=T):
    src = np.maximum((np.arange(out_size, dtype=np.float64) + 0.5) * (L / out_size) - 0.5, 0.0)
    i0 = np.clip(np.floor(src).astype(np.int64), 0, L - 1)
    i1 = np.minimum(i0 + 1, L - 1)
    w = src - i0
    U = np.zeros((out_size, L), np.float64)
    U[np.arange(out_size), i0] += 1.0 - w
    U[np.arange(out_size), i1] += w
    return U.astype(np.float32)


def make_plan():
    """Input-value-independent schedule + interp weight blocks."""
    # DWT schedule: merged low+high groups of 64 windows: (g, is_last)
    dwt = []
    for lvl in range(LEVELS):
        nw = NWS[lvl]
        ng = (nw + 63) // 64
        dwt.append([(g, g == ng - 1) for g in range(ng)])

    # interp schedule: blocks consumed in order; per chunk: (s, kt, K, idx)
    Ls = [NWS[0], NWS[1], NWS[2], NWS[2]]
    Us = [_interp_mat(L) for L in Ls]
    ublks = []
    isched = [[] for _ in range(NCH)]
    for c in range(NCH):
        for s in range(4):
            U, L = Us[s], Ls[s]
            cols = U[512 * c:512 * (c + 1)]           # [512, L]
            nz = np.nonzero(cols.any(0))[0]
            for kt in range(nz.min() // P, nz.max() // P + 1):
                K = min(P, L - P * kt)
                blk = cols[:, P * kt:P * kt + K].T    # [K, 512]
                if not np.any(blk):
                    continue
                full = np.zeros((P, 512), np.float32)
                full[:K] = blk
                isched[c].append((s, kt, K, len(ublks)))
                ublks.append(full)
    ublk = np.stack(ublks)                            # [NB, 128, 512] f32
    return {"dwt": dwt, "isched": isched, "ublk": ublk, "nb": len(ublks)}


def _toeplitz(nw, Lp, f):
    F = np.zeros((nw, Lp), np.float32)
    for w in range(nw):
        F[w, 2 * w:2 * w + 4] = f
    return F


def make_consts(inputs, plan):
    """Host-side constants (depend on input values)."""
    h0, h1 = np.asarray(inputs["h0"]), np.asarray(inputs["h1"])
    f0 = h0[:, 0, :, 0].astype(np.float32)
    f1 = h1[:, 0, :, 0].astype(np.float32)
    ln1_g = np.asarray(inputs["ln1_g"], np.float32)
    ln1_b = np.asarray(inputs["ln1_b"], np.float32)
    ln2_g = np.asarray(inputs["ln2_g"], np.float32)
    ln2_b = np.asarray(inputs["ln2_b"], np.float32)
    proj_w = np.asarray(inputs["proj_w"], np.float32)
    proj_b = np.asarray(inputs["proj_b"], np.float32)
    w1 = np.asarray(inputs["w1"], np.float32)
    b1 = np.asarray(inputs["b1"], np.float32)
    w2 = np.asarray(inputs["w2"], np.float32)
    b2 = np.asarray(inputs["b2"], np.float32)

    # merged DWT lhsT blocks [9,128,128]: cols 0..63 low (f0), 64..127 high (f1)
    fblk = np.zeros((9, P, P), np.float32)
    for lvl in range(LEVELS):
        A = fblk[lvl * 3 + 0]
        for r in range(P):
            for w in range(64):
                k = r - 2 * w
                if 0 <= k < 4:
                    A[r, w] = f0[lvl][k]
                    A[r, 64 + w] = f1[lvl][k]
        Bt = fblk[lvl * 3 + 1]
        for r in range(2):
            Bt[r, 63] = f0[lvl][r + 2]
            Bt[r, 127] = f1[lvl][r + 2]
        Al = fblk[lvl * 3 + 2]
        Al[:] = A
        Al[:, 63] = 0.0
        Al[:, 127] = 0.0

    # m1 = wavelet operator applied to ones(T) (for ln1_b fold)
    ones = np.ones((T, 1), np.float32)
    a = ones
    comb = np.zeros((T, 1), np.float32)
    Us = [_interp_mat(L) for L in [NWS[0], NWS[1], NWS[2], NWS[2]]]
    for lvl in range(LEVELS):
        ap = np.zeros((LPADS[lvl], 1), np.float32)
        ap[:a.shape[0]] = a
        comb += Us[lvl] @ (_toeplitz(NWS[lvl], LPADS[lvl], f1[lvl]) @ ap)
        a = _toeplitz(NWS[lvl], LPADS[lvl], f0[lvl]) @ ap
    comb += Us[3] @ a
    m1 = comb[:, 0]                               # [T]

    wg = (ln1_g[:, None] * proj_w)                # LN1 g fold
    bW = ln1_b @ proj_w                           # LN1 b fold (rank-1 with m1)
    w1g = (ln2_g[:, None] * w1)                   # LN2 g fold
    b1f = b1 + ln2_b @ w1                         # LN2 b fold

    return {
        "wg": wg.astype(BF16),
        "w1": w1g.astype(BF16),
        "w2": w2.astype(BF16),
        "fblk": fblk.reshape(9 * P, P).astype(BF16),       # [9*128, 128]
        "ublk": plan["ublk"].reshape(-1, 512).astype(BF16),  # [NB*128, 512]
        "b1c": np.ascontiguousarray(b1f.reshape(NFT, P).T.astype(np.float32)),  # [128,16]
        "r1l": np.stack([np.ones(T, np.float32), m1]).astype(BF16),  # [2, T] (ones, m1)
        "r1r": np.stack([proj_b, bW]).astype(BF16),                  # [2, 512]
        "b2r": b2.reshape(1, D).astype(BF16),                        # [1, 512]
        "idn": np.identity(P, np.float32).astype(BF16),              # [128,128]
    }


# ----------------------------------------------------------------- bass
def build_nc(plan):
    import concourse.bacc as bacc
    import concourse.tile as tile
    from concourse import mybir

    F32, BF = mybir.dt.float32, mybir.dt.bfloat16
    AF = mybir.ActivationFunctionType
    OP = mybir.AluOpType

    nc = bacc.Bacc("TRN2", target_bir_lowering=False, debug=False, name="daub")
    x_d = nc.dram_tensor("x", [T, D], F32, kind="ExternalInput")
    out_d = nc.dram_tensor("out", [T, D], F32, kind="ExternalOutput")
    wg_d = nc.dram_tensor("wg", [D, D], BF, kind="ExternalInput")
    w1_d = nc.dram_tensor("w1", [D, FFN], BF, kind="ExternalInput")
    w2_d = nc.dram_tensor("w2", [FFN, D], BF, kind="ExternalInput")
    fblk_d = nc.dram_tensor("fblk", [9 * P, P], BF, kind="ExternalInput")
    ublk_d = nc.dram_tensor("ublk", [plan["nb"] * P, 512], BF, kind="ExternalInput")
    b1c_d = nc.dram_tensor("b1c", [P, NFT], F32, kind="ExternalInput")
    r1l_d = nc.dram_tensor("r1l", [2, T], BF, kind="ExternalInput")
    r1r_d = nc.dram_tensor("r1r", [2, D], BF, kind="ExternalInput")
    b2r_d = nc.dram_tensor("b2r", [1, D], BF, kind="ExternalInput")
    idn_d = nc.dram_tensor("idn", [P, P], BF, kind="ExternalInput")

    with tile.TileContext(nc) as tc:
        import contextlib
        ctx = contextlib.ExitStack()
        pw = ctx.enter_context(tc.tile_pool(name="pw", bufs=1))
        pbig = ctx.enter_context(tc.tile_pool(name="pbig", bufs=1))
        pio = ctx.enter_context(tc.tile_pool(name="pio", bufs=3))
        pu = ctx.enter_context(tc.tile_pool(name="pu", bufs=16))
        px2 = ctx.enter_context(tc.tile_pool(name="px2", bufs=8))
        pcomb = ctx.enter_context(tc.tile_pool(name="pcomb", bufs=2))
        pxn2 = ctx.enter_context(tc.tile_pool(name="pxn2", bufs=2))
        pst = ctx.enter_context(tc.tile_pool(name="pst", bufs=1))
        ptm = ctx.enter_context(tc.tile_pool(name="ptm", bufs=8))
        ppace = ctx.enter_context(tc.tile_pool(name="ppace", bufs=1))
        ps_i = ctx.enter_context(tc.tile_pool(name="ps_i", bufs=3, space="PSUM"))
        ps_n = ctx.enter_context(tc.tile_pool(name="ps_n", bufs=2, space="PSUM"))
        ps_p = ctx.enter_context(tc.tile_pool(name="ps_p", bufs=1, space="PSUM"))
        ps_h = ctx.enter_context(tc.tile_pool(name="ps_h", bufs=2, space="PSUM"))

        # ---- small consts first (gate warmup + DWT)
        idn_sb = pw.tile([P, P], BF, name="idn_sb")
        nc.sync.dma_start(out=idn_sb, in_=idn_d[:, :])
        fblk_sb = pw.tile([P, 9, P], BF, name="fblk_sb")
        nc.sync.dma_start(out=fblk_sb, in_=fblk_d.rearrange("(b p) m -> p b m", p=P))
        b1c_sb = pw.tile([P, NFT], F32, name="b1c_sb")
        nc.sync.dma_start(out=b1c_sb, in_=b1c_d[:, :])
        r1l_sb = pw.tile([2, T], BF, name="r1l_sb")
        nc.sync.dma_start(out=r1l_sb, in_=r1l_d[:, :])
        r1r_sb = pw.tile([2, D], BF, name="r1r_sb")
        nc.sync.dma_start(out=r1r_sb, in_=r1r_d[:, :])
        b2r_sb = pw.tile([1, D], BF, name="b2r_sb")
        nc.sync.dma_start(out=b2r_sb, in_=b2r_d[:, :])
        eps_sb = pw.tile([P, 1], F32, name="eps_sb")
        nc.vector.memset(eps_sb, EPS)
        wg_sb = pw.tile([P, NDT, D], BF, name="wg_sb")
        w1_sb = pw.tile([P, NDT, FFN], BF, name="w1_sb")
        w2_sb = pw.tile([P, NFT, D], BF, name="w2_sb")

        # ---- HAM pacer: serial (SWDGE dma -> tiny MM) chain drips PE activity
        # (~1-2us per link) through the DMA/LN1-bound lead-in so the PE clock
        # gate stays at 8/8 when the real matmul stream begins.
        wups = ps_h.tile([P, P], F32, tag="ps_h", name="wups")
        for wi in range(96):
            nc.tensor.matmul(wups, idn_sb, idn_sb, start=(wi == 0), stop=(wi == 95))
        wud = pw.tile([P, 1], F32, name="wud")
        nc.vector.tensor_copy(out=wud, in_=wups[:, 0:1])

        # ---- big activations
        xh_sb = pbig.tile([P, NT, D], BF, name="xh_sb")      # xn, later hdn
        a1 = pbig.tile([P, 16, D], BF, name="a1")
        a2 = pbig.tile([P, 8, D], BF, name="a2")
        d0 = pbig.tile([P, 16, D], BF, name="d0")
        mu2_sb = pst.tile([P, NT], F32, name="mu2_sb")
        rs2_sb = pst.tile([P, NT], F32, name="rs2_sb")

        # ---------------- P1: LN1 -> xn (bf16)
        for i in range(NT):
            xt = pio.tile([P, D], F32, tag="xt", name=f"xt{i}")
            nc.sync.dma_start(out=xt, in_=x_d[P * i:P * (i + 1), :])
            st = pio.tile([P, 6], F32, tag="st", name=f"st{i}")
            nc.vector.bn_stats(out=st, in_=xt)
            mv = pio.tile([P, 2], F32, tag="mv", name=f"mv{i}")
            nc.vector.bn_aggr(out=mv, in_=st)
            sd = pio.tile([P, 1], F32, tag="sd", name=f"sd{i}")
            nc.scalar.activation(out=sd, in_=mv[:, 1:2], func=AF.Sqrt, bias=eps_sb)
            nc.vector.reciprocal(out=sd, in_=sd)
            nc.vector.tensor_scalar(
                out=xh_sb[:, i, :], in0=xt, scalar1=mv[:, 0:1], scalar2=sd,
                op0=OP.subtract, op1=OP.mult)
            wt_ = ps_h.tile([P, P], F32, tag="ps_h", name=f"wu{i}")
            nc.tensor.matmul(wt_, idn_sb, xh_sb[:, i, 0:P], start=True, stop=True)

        # ---------------- P2: DWT cascade (merged low+high matmuls)
        # zero pad rows (2047 of a1-input, 1023 of a2-input) before evacs
        nc.vector.memset(a1[96:128, 15, :], 0.0)
        nc.vector.memset(a2[96:128, 7, :], 0.0)
        # aliases: d1 -> a1[0:8], d2 -> a2[0:4], a3 -> a1[8:12]
        srcs = [(xh_sb, 0), (a1, 0), (a2, 0)]
        lows = [(a1, 0), (a2, 0), (a1, 8)]
        highs = [(d0, 0), (a1, 0), (a2, 0)]
        for lvl in range(LEVELS):
            (src, sb), (low, lb), (high, hb) = srcs[lvl], lows[lvl], highs[lvl]
            for (g, last) in plan["dwt"][lvl]:
                pst_ = ps_i.tile([P, D], F32, tag="psA", name=f"dw{lvl}_{g}")
                nc.tensor.matmul(
                    pst_, fblk_sb[:, lvl * 3 + (2 if last else 0), :],
                    src[:, sb + g, :], start=True, stop=last)
                if not last:
                    nc.tensor.matmul(
                        pst_, fblk_sb[:2, lvl * 3 + 1, :], src[:2, sb + g + 1, :],
                        start=False, stop=True)
                Mg = 63 if last else 64
                lo = 64 * (g % 2)
                nc.scalar.copy(out=low[lo:lo + Mg, lb + g // 2, :], in_=pst_[0:Mg, :])
                nc.vector.tensor_copy(out=high[lo:lo + Mg, hb + g // 2, :],
                                      in_=pst_[64:64 + Mg, :])

        # GEMM weights now (DMA overlaps interp/proj phases)
        nc.sync.dma_start(out=wg_sb, in_=wg_d.rearrange("(kt p) n -> p kt n", p=P))
        nc.sync.dma_start(out=w1_sb, in_=w1_d.rearrange("(kt p) n -> p kt n", p=P))
        nc.sync.dma_start(out=w2_sb, in_=w2_d.rearrange("(kt p) n -> p kt n", p=P))

        # ------- P3: per t-chunk: interp -> proj+residual -> LN2 -> FFN -> out
        dsrc = [(d0, 0), (a1, 0), (a2, 0), (a1, 8)]
        for c in range(NCH):
            ub = {}
            for (s, kt, K, idx) in plan["isched"][c]:
                ut = pu.tile([P, 512], BF, tag="ub", name=f"ub{idx}")
                nc.sync.dma_start(out=ut, in_=ublk_d[P * idx:P * (idx + 1), :])
                ub[idx] = ut
            comb_c = pcomb.tile([P, NDT, 512], BF, tag="comb", name=f"comb{c}")
            combp = pcomb.tile([P, NDT, 512], BF, tag="combp", name=f"combp{c}")
            sch1 = [b for b in plan["isched"][c] if b[0] < 2]
            sch2 = [b for b in plan["isched"][c] if b[0] >= 2]
            for dt in range(NDT):
                psi = ps_n.tile([P, 512], F32, tag="ps_int", name=f"pi{c}_{dt}")
                for q, (s, kt, K, idx) in enumerate(sch1):
                    dt_, db_ = dsrc[s]
                    nc.tensor.matmul(
                        psi, dt_[:K, db_ + kt, P * dt:P * (dt + 1)], ub[idx][:K, :],
                        start=(q == 0), stop=(q == len(sch1) - 1))
                nc.vector.tensor_copy(out=combp[:, dt, :], in_=psi)
            for dt in range(NDT):
                psi = ps_i.tile([P, 512], F32, tag="psA", name=f"pj{c}_{dt}")
                for q, (s, kt, K, idx) in enumerate(sch2):
                    dt_, db_ = dsrc[s]
                    nc.tensor.matmul(
                        psi, dt_[:K, db_ + kt, P * dt:P * (dt + 1)], ub[idx][:K, :],
                        start=(q == 0), stop=(q == len(sch2) - 1))
                nc.vector.scalar_tensor_tensor(
                    out=comb_c[:, dt, :], in0=psi, scalar=1.0,
                    in1=combp[:, dt, :], op0=OP.mult, op1=OP.add)
            x2ts = []
            for tj in range(4):
                ti = 4 * c + tj
                psp = ps_p.tile([P, D], F32, tag="ps_pt", name=f"pp{ti}")
                for dt in range(NDT):
                    nc.tensor.matmul(
                        psp, comb_c[:, dt, P * tj:P * (tj + 1)], wg_sb[:, dt, :],
                        start=(dt == 0), stop=False)
                nc.tensor.matmul(
                    psp, r1l_sb[:, P * ti:P * (ti + 1)], r1r_sb[:, :],
                    start=False, stop=True)
                xt = pio.tile([P, D], F32, tag="xt", name=f"xr{ti}")
                nc.sync.dma_start(out=xt, in_=x_d[P * ti:P * (ti + 1), :])
                x2t = px2.tile([P, D], F32, tag="x2t", name=f"x2t{ti}")
                nc.vector.tensor_add(out=x2t, in0=psp, in1=xt)
                x2ts.append(x2t)
                st = pio.tile([P, 6], F32, tag="st", name=f"st2_{ti}")
                nc.vector.bn_stats(out=st, in_=x2t)
                mv = pio.tile([P, 2], F32, tag="mv", name=f"mv2_{ti}")
                nc.vector.bn_aggr(out=mv, in_=st)
                nc.vector.tensor_copy(out=mu2_sb[:, ti:ti + 1], in_=mv[:, 0:1])
                sd = pio.tile([P, 1], F32, tag="sd", name=f"sd2_{ti}")
                nc.scalar.activation(out=sd, in_=mv[:, 1:2], func=AF.Sqrt, bias=eps_sb)
                nc.vector.reciprocal(out=rs2_sb[:, ti:ti + 1], in_=sd)
            # LN2 apply + transpose -> xn2 (fm)
            xn2f = pxn2.tile([P, NDT, 512], BF, tag="xn2f", name=f"xn2f{c}")
            tmts = []
            for tj in range(4):
                ti = 4 * c + tj
                tmt = ptm.tile([P, D], BF, tag="tmt", name=f"tmt{ti}")
                nc.vector.tensor_scalar(
                    out=tmt, in0=x2ts[tj], scalar1=mu2_sb[:, ti:ti + 1],
                    scalar2=rs2_sb[:, ti:ti + 1], op0=OP.subtract, op1=OP.mult)
                tmts.append(tmt)
            for dt in range(NDT):
                pstp = ps_p.tile([P, 512], BF, tag="ps_pt", name=f"pt{c}_{dt}")
                for tj in range(4):
                    nc.tensor.transpose(
                        pstp[:, P * tj:P * (tj + 1)],
                        tmts[tj][:, P * dt:P * (dt + 1)], idn_sb)
                nc.scalar.copy(out=xn2f[:, dt, :], in_=pstp)
            # FFN1 + gelu -> hdn in xh_sb slots
            hbase = NFT * (c % 2)
            for ft in range(NFT):
                psh = ps_h.tile([P, 512], F32, tag="ps_h", name=f"ph{c}_{ft}")
                for dt in range(NDT):
                    nc.tensor.matmul(
                        psh, w1_sb[:, dt, P * ft:P * (ft + 1)], xn2f[:, dt, :],
                        start=(dt == 0), stop=(dt == NDT - 1))
                nc.scalar.activation(
                    out=xh_sb[:, hbase + ft, :], in_=psh, func=AF.Gelu,
                    bias=b1c_sb[:, ft:ft + 1])
            # FFN2 + residual -> out
            for tj in range(4):
                ti = 4 * c + tj
                pso = ps_i.tile([P, D], F32, tag="psA", name=f"po{ti}")
                for kt in range(NFT):
                    nc.tensor.matmul(
                        pso, xh_sb[:, hbase + kt, P * tj:P * (tj + 1)], w2_sb[:, kt, :],
                        start=(kt == 0), stop=False)
                nc.tensor.matmul(
                    pso, r1l_sb[0:1, P * ti:P * (ti + 1)], b2r_sb[:, :],
                    start=False, stop=True)
                ot = pio.tile([P, D], F32, tag="ot", name=f"ot{ti}")
                nc.vector.tensor_add(out=ot, in0=pso, in1=x2ts[tj])
                nc.sync.dma_start(out=out_d[P * ti:P * (ti + 1), :], in_=ot)
        ctx.close()
    nc.compile()
    return nc


_BUILT = {}


def _get_built():
    if "nc" not in _BUILT:
        plan = make_plan()
        _BUILT["plan"] = plan
        _BUILT["nc"] = build_nc(plan)
    return _BUILT["nc"], _BUILT["plan"]


def kernel(**inputs):
    from concourse.bass_utils import run_bass_kernel_spmd

    nc, plan = _get_built()
    consts = make_consts(inputs, plan)
    x = np.ascontiguousarray(np.asarray(inputs["x"], np.float32))
    in_maps = []
    for b in range(B):
        m = {"x": np.ascontiguousarray(x[b])}
        m.update(consts)
        in_maps.append(m)
    res = run_bass_kernel_spmd(nc, in_maps, core_ids=list(range(B)))
    out = np.stack([res.results[b]["out"] for b in range(B)]).astype(np.float32)
    return out



# revision 2
# speedup vs baseline: 1.6140x; 1.6140x over previous
"""Trainium2 Bass kernel for nn_MultiHeadDaubechiesBlock.

Data-parallel over batch B=8 across 8 NeuronCores (one sequence per core).

The whole DWT cascade + linear-interp upsample + sum is a fixed linear
operator A [T,T] on the token axis, identical for every channel/head
(the Daubechies filters are broadcast across heads/channels in this
module). A is built host-side (sparse, banded: ~30-wide rows) from the
runtime h0/h1 values and applied on-device as banded 128x512 matmuls:
  combined_fm[c, t'] = sum_t xn[t, c] * A[t', t]
which directly yields the feature-major layout the proj GEMM needs.

Per-core pipeline (chunked by 512 tokens, software-pipelined):
  LN1 (DVE bn_stats, token-major, g/b folded into proj weights)
  -> A-apply (banded matmuls, bf16)
  -> proj GEMM + rank-2 bias/LN-fold + residual -> x2
  -> LN2 stats -> normalize -> PE transpose to feature-major (fp8)
  -> FFN1 fp8 DoubleRow + exact gelu (ACT, scale+bias fold) -> hdn fp8
  -> FFN2 fp8 DoubleRow + rank-1 b2 (bf16 mixed into same PSUM group)
  -> + residual -> out.
fp8 GEMM weights are pre-scaled x512 host-side; the 1/512 is folded
into the ACT/DVE evacuations.
"""
import numpy as np
import ml_dtypes

B, T, D, H, DH, LEVELS, FFN = 8, 4096, 512, 4, 128, 3, 2048
P = 128
NT = T // P          # 32 token tiles
NDT = D // P         # 4 feature tiles
NFT = FFN // P       # 16 ffn tiles
NCH = 8              # t-chunks of 512
EPS = 1e-5
BF16 = ml_dtypes.bfloat16
F8 = ml_dtypes.float8_e4m3
FSCALE = 512.0       # fp8 weight pre-scale


# ----------------------------------------------------------------- host
def _dwt_sp(L, f):
    import scipy.sparse as sp
    Lp = max(L, 4)
    if (Lp - 4) % 2 != 0:
        Lp += 1
    nw = (Lp - 4) // 2 + 1
    rows, cols, vals = [], [], []
    w = np.arange(nw)
    for k in range(4):
        c = 2 * w + k
        m = c < L
        rows.append(w[m])
        cols.append(c[m])
        vals.append(np.full(int(m.sum()), f[k], np.float64))
    return sp.csr_matrix(
        (np.concatenate(vals), (np.concatenate(rows), np.concatenate(cols))),
        shape=(nw, L))


def _interp_sp(L, out=T):
    import scipy.sparse as sp
    src = np.maximum((np.arange(out) + 0.5) * (L / out) - 0.5, 0.0)
    i0 = np.clip(np.floor(src).astype(np.int64), 0, L - 1)
    i1 = np.minimum(i0 + 1, L - 1)
    w = src - i0
    r = np.concatenate([np.arange(out), np.arange(out)])
    c = np.concatenate([i0, i1])
    v = np.concatenate([1.0 - w, w])
    return sp.csr_matrix((v, (r, c)), shape=(out, L))


def _build_A(f0s, f1s):
    """A [T,T]: combined = A @ xn (per channel)."""
    import scipy.sparse as sp
    A = None
    W = sp.identity(T, format="csr")
    L = T
    for lvl in range(LEVELS):
        det = _dwt_sp(L, f1s[lvl]) @ W
        W = _dwt_sp(L, f0s[lvl]) @ W
        term = _interp_sp(det.shape[0]) @ det
        A = term if A is None else A + term
        L = W.shape[0]
    return A + _interp_sp(L) @ W


def make_plan():
    """Input-value-independent: band structure from all-ones filters
    (support superset of any filter values)."""
    ones4 = np.ones(4)
    A1 = _build_A([ones4] * LEVELS, [ones4] * LEVELS)
    band = []     # per chunk: list of (kt, bidx)
    nb = 0
    for c in range(NCH):
        rows = np.abs(A1[512 * c:512 * (c + 1)])
        colmax = np.asarray(rows.max(0).todense())[0]
        nz = np.nonzero(colmax > 0)[0]
        kts = sorted(set(nz // P))
        band.append([(int(kt), nb + q) for q, kt in enumerate(kts)])
        nb += len(kts)
    return {"band": band, "nb": nb}


def make_consts(inputs, plan):
    h0, h1 = np.asarray(inputs["h0"]), np.asarray(inputs["h1"])
    f0 = h0[:, 0, :, 0].astype(np.float64)
    f1 = h1[:, 0, :, 0].astype(np.float64)
    ln1_g = np.asarray(inputs["ln1_g"], np.float32)
    ln1_b = np.asarray(inputs["ln1_b"], np.float32)
    ln2_g = np.asarray(inputs["ln2_g"], np.float32)
    ln2_b = np.asarray(inputs["ln2_b"], np.float32)
    proj_w = np.asarray(inputs["proj_w"], np.float32)
    proj_b = np.asarray(inputs["proj_b"], np.float32)
    w1 = np.asarray(inputs["w1"], np.float32)
    b1 = np.asarray(inputs["b1"], np.float32)
    w2 = np.asarray(inputs["w2"], np.float32)
    b2 = np.asarray(inputs["b2"], np.float32)

    A = _build_A(list(f0), list(f1)).tocsc()
    # banded A^T blocks [nb, 128, 512] (rhs of the A-apply matmuls)
    atb = np.zeros((plan["nb"], P, 512), np.float32)
    for c in range(NCH):
        for kt, bidx in plan["band"][c]:
            blk = A[512 * c:512 * (c + 1), P * kt:P * (kt + 1)]
            atb[bidx] = np.asarray(blk.todense()).T
    m1 = np.asarray(A @ np.ones(T))            # A @ 1 (for ln1_b fold)

    wg = ln1_g[:, None] * proj_w               # LN1 g fold
    bW = ln1_b @ proj_w                        # LN1 b fold (rank-1 with m1)
    w1g = ln2_g[:, None] * w1                  # LN2 g fold
    b1f = b1 + ln2_b @ w1                      # LN2 b fold

    def fp8(a):
        return np.clip(a, -240, 240).astype(F8)

    return {
        "wg": wg.astype(BF16),
        "w1": fp8(w1g * FSCALE),                                  # [D, FFN]
        "w2": fp8(w2 * FSCALE),                                   # [FFN, D]
        "atb": atb.reshape(plan["nb"] * P, 512).astype(BF16),
        "b1c": np.ascontiguousarray(b1f.reshape(NFT, P).T.astype(np.float32)),
        "r1l": np.stack([np.ones(T, np.float32), m1]).astype(BF16),  # [2, T]
        "r1r": np.stack([proj_b, bW]).astype(BF16),                  # [2, D]
        "b2r": (b2 * FSCALE).reshape(1, D).astype(BF16),             # [1, D]
        "idn": np.identity(P, np.float32).astype(BF16),              # [P, P]
    }


# ----------------------------------------------------------------- bass
def build_nc(plan):
    import concourse.bacc as bacc
    import concourse.tile as tile
    from concourse import mybir

    F32, BF, E4 = mybir.dt.float32, mybir.dt.bfloat16, mybir.dt.float8e4
    AF = mybir.ActivationFunctionType
    OP = mybir.AluOpType
    PM = mybir.MatmulPerfMode
    NB = plan["nb"]

    nc = bacc.Bacc("TRN2", target_bir_lowering=False, debug=False, name="daub")
    x_d = nc.dram_tensor("x", [T, D], F32, kind="ExternalInput")
    out_d = nc.dram_tensor("out", [T, D], F32, kind="ExternalOutput")
    wg_d = nc.dram_tensor("wg", [D, D], BF, kind="ExternalInput")
    w1_d = nc.dram_tensor("w1", [D, FFN], E4, kind="ExternalInput")
    w2_d = nc.dram_tensor("w2", [FFN, D], E4, kind="ExternalInput")
    atb_d = nc.dram_tensor("atb", [NB * P, 512], BF, kind="ExternalInput")
    b1c_d = nc.dram_tensor("b1c", [P, NFT], F32, kind="ExternalInput")
    r1l_d = nc.dram_tensor("r1l", [2, T], BF, kind="ExternalInput")
    r1r_d = nc.dram_tensor("r1r", [2, D], BF, kind="ExternalInput")
    b2r_d = nc.dram_tensor("b2r", [1, D], BF, kind="ExternalInput")
    idn_d = nc.dram_tensor("idn", [P, P], BF, kind="ExternalInput")

    with tile.TileContext(nc) as tc:
        import contextlib
        ctx = contextlib.ExitStack()
        pw = ctx.enter_context(tc.tile_pool(name="pw", bufs=1))
        pbig = ctx.enter_context(tc.tile_pool(name="pbig", bufs=1))
        pio = ctx.enter_context(tc.tile_pool(name="pio", bufs=4))
        pcomb = ctx.enter_context(tc.tile_pool(name="pcomb", bufs=2))
        px2 = ctx.enter_context(tc.tile_pool(name="px2", bufs=8))
        ptm = ctx.enter_context(tc.tile_pool(name="ptm", bufs=8))
        pxn2 = ctx.enter_context(tc.tile_pool(name="pxn2", bufs=2))
        phd = ctx.enter_context(tc.tile_pool(name="phd", bufs=2))
        pst = ctx.enter_context(tc.tile_pool(name="pst", bufs=1))
        ps_a = ctx.enter_context(tc.tile_pool(name="ps_a", bufs=2, space="PSUM"))
        ps_p = ctx.enter_context(tc.tile_pool(name="ps_p", bufs=2, space="PSUM"))
        ps_h = ctx.enter_context(tc.tile_pool(name="ps_h", bufs=2, space="PSUM"))
        ps_o = ctx.enter_context(tc.tile_pool(name="ps_o", bufs=2, space="PSUM"))

        # ---- small consts
        idn_sb = pw.tile([P, P], BF, name="idn_sb")
        nc.sync.dma_start(out=idn_sb, in_=idn_d[:, :])
        b1c_sb = pw.tile([P, NFT], F32, name="b1c_sb")
        nc.sync.dma_start(out=b1c_sb, in_=b1c_d[:, :])
        r1l_sb = pw.tile([2, T], BF, name="r1l_sb")
        nc.sync.dma_start(out=r1l_sb, in_=r1l_d[:, :])
        r1r_sb = pw.tile([2, D], BF, name="r1r_sb")
        nc.sync.dma_start(out=r1r_sb, in_=r1r_d[:, :])
        b2r_sb = pw.tile([1, D], BF, name="b2r_sb")
        nc.sync.dma_start(out=b2r_sb, in_=b2r_d[:, :])
        eps_sb = pw.tile([P, 1], F32, name="eps_sb")
        nc.vector.memset(eps_sb, EPS)

        # ---- A blocks: per-chunk DMAs so chunk 0 isn't gated on the rest
        atb_sb = pw.tile([P, NB, 512], BF, name="atb_sb")
        atb_r = atb_d.rearrange("(b p) n -> p b n", p=P)
        for c in range(NCH):
            b0 = plan["band"][c][0][1]
            b1_ = plan["band"][c][-1][1] + 1
            nc.sync.dma_start(out=atb_sb[:, b0:b1_, :], in_=atb_r[:, b0:b1_, :])

        # ---- GEMM weights
        wg_sb = pw.tile([P, NDT, D], BF, name="wg_sb")
        nc.sync.dma_start(out=wg_sb, in_=wg_d.rearrange("(kt p) n -> p kt n", p=P))
        w1_sb = pw.tile([P, NDT, FFN], E4, name="w1_sb")
        nc.sync.dma_start(out=w1_sb, in_=w1_d.rearrange("(kt p) n -> p kt n", p=P))
        w2_sb = pw.tile([P, NFT, D], E4, name="w2_sb")
        nc.sync.dma_start(out=w2_sb, in_=w2_d.rearrange("(kt p) n -> p kt n", p=P))

        # ---- HAM pacer: serial matmul chain bridges the LN1 lead-in so
        # the PE clock gate is at 8/8 when the real matmul stream begins.
        wups = ps_h.tile([P, P], F32, tag="ps_h", name="wups")
        for wi in range(96):
            nc.tensor.matmul(wups, idn_sb, idn_sb, start=(wi == 0), stop=(wi == 95))
        wud = pw.tile([P, 1], F32, name="wud")
        nc.vector.tensor_copy(out=wud, in_=wups[:, 0:1])

        # ---- big activations
        xn_sb = pbig.tile([P, NT, D], BF, name="xn_sb")
        mu2_sb = pst.tile([P, NT], F32, name="mu2_sb")
        rs2_sb = pst.tile([P, NT], F32, name="rs2_sb")

        def ln1_tile(i):
            xt = pio.tile([P, D], F32, tag="xt", name=f"xt{i}")
            nc.sync.dma_start(out=xt, in_=x_d[P * i:P * (i + 1), :])
            st = pio.tile([P, 6], F32, tag="st", name=f"st{i}")
            nc.vector.bn_stats(out=st, in_=xt)
            mv = pio.tile([P, 2], F32, tag="mv", name=f"mv{i}")
            nc.vector.bn_aggr(out=mv, in_=st)
            sd = pio.tile([P, 1], F32, tag="sd", name=f"sd{i}")
            nc.scalar.activation(out=sd, in_=mv[:, 1:2], func=AF.Sqrt, bias=eps_sb)
            nc.vector.reciprocal(out=sd, in_=sd)
            nc.vector.tensor_scalar(
                out=xn_sb[:, i, :], in0=xt, scalar1=mv[:, 0:1], scalar2=sd,
                op0=OP.subtract, op1=OP.mult)

        for i in range(6):
            ln1_tile(i)

        for c in range(NCH):
            # ---- A-apply: combined (feature-major) for this chunk
            comb = pcomb.tile([P, NDT, 512], BF, tag="comb", name=f"comb{c}")
            for dt in range(NDT):
                psA = ps_a.tile([P, 512], F32, tag="ps_a", name=f"pa{c}_{dt}")
                nq = len(plan["band"][c])
                for q, (kt, bidx) in enumerate(plan["band"][c]):
                    nc.tensor.matmul(
                        psA, xn_sb[:, kt, P * dt:P * (dt + 1)],
                        atb_sb[:, bidx, :], start=(q == 0), stop=(q == nq - 1))
                nc.scalar.copy(out=comb[:, dt, :], in_=psA)

            # ---- proj + residual + LN2 stats
            x2ts = []
            for tj in range(4):
                ti = 4 * c + tj
                psp = ps_p.tile([P, D], F32, tag="ps_p", name=f"pp{ti}")
                for dt in range(NDT):
                    nc.tensor.matmul(
                        psp, comb[:, dt, P * tj:P * (tj + 1)], wg_sb[:, dt, :],
                        start=(dt == 0), stop=False)
                nc.tensor.matmul(
                    psp, r1l_sb[:, P * ti:P * (ti + 1)], r1r_sb[:, :],
                    start=False, stop=True)
                xt = pio.tile([P, D], F32, tag="xt", name=f"xr{ti}")
                nc.sync.dma_start(out=xt, in_=x_d[P * ti:P * (ti + 1), :])
                x2t = px2.tile([P, D], F32, tag="x2t", name=f"x2t{ti}")
                nc.vector.tensor_add(out=x2t, in0=psp, in1=xt)
                x2ts.append(x2t)
                st = pio.tile([P, 6], F32, tag="st", name=f"st2_{ti}")
                nc.vector.bn_stats(out=st, in_=x2t)
                mv = pio.tile([P, 2], F32, tag="mv", name=f"mv2_{ti}")
                nc.vector.bn_aggr(out=mv, in_=st)
                nc.vector.tensor_copy(out=mu2_sb[:, ti:ti + 1], in_=mv[:, 0:1])
                sd = pio.tile([P, 1], F32, tag="sd", name=f"sd2_{ti}")
                nc.scalar.activation(out=sd, in_=mv[:, 1:2], func=AF.Sqrt, bias=eps_sb)
                nc.vector.reciprocal(out=rs2_sb[:, ti:ti + 1], in_=sd)

            # ---- LN2 apply + transpose -> xn2 (feature-major, fp8)
            xn2f = pxn2.tile([P, NDT, 512], E4, tag="xn2f", name=f"xn2f{c}")
            tmts = []
            for tj in range(4):
                ti = 4 * c + tj
                tmt = ptm.tile([P, D], BF, tag="tmt", name=f"tmt{ti}")
                nc.vector.tensor_scalar(
                    out=tmt, in0=x2ts[tj], scalar1=mu2_sb[:, ti:ti + 1],
                    scalar2=rs2_sb[:, ti:ti + 1], op0=OP.subtract, op1=OP.mult)
                tmts.append(tmt)
            for dt in range(NDT):
                pstp = ps_p.tile([P, 512], BF, tag="ps_p", name=f"pt{c}_{dt}")
                for tj in range(4):
                    nc.tensor.transpose(
                        pstp[:, P * tj:P * (tj + 1)],
                        tmts[tj][:, P * dt:P * (dt + 1)], idn_sb)
                nc.scalar.copy(out=xn2f[:, dt, :], in_=pstp)

            # ---- FFN1 fp8 DoubleRow + gelu -> hdn fp8
            hdn = phd.tile([P, NFT, 512], E4, tag="hdn", name=f"hdn{c}")
            for ft in range(NFT):
                psh = ps_h.tile([P, 512], F32, tag="ps_h", name=f"ph{c}_{ft}")
                for q in range(2):
                    nc.tensor.matmul(
                        psh, w1_sb[:, 2 * q:2 * q + 2, P * ft:P * (ft + 1)],
                        xn2f[:, 2 * q:2 * q + 2, :],
                        start=(q == 0), stop=(q == 1), perf_mode=PM.DoubleRow)
                nc.scalar.activation(
                    out=hdn[:, ft, :], in_=psh, func=AF.Gelu,
                    bias=b1c_sb[:, ft:ft + 1], scale=1.0 / FSCALE)

            # ---- FFN2 fp8 DoubleRow + rank-1 b2 + residual -> out
            for tj in range(4):
                ti = 4 * c + tj
                pso = ps_o.tile([P, D], F32, tag="ps_o", name=f"po{ti}")
                for q in range(NFT // 2):
                    nc.tensor.matmul(
                        pso, hdn[:, 2 * q:2 * q + 2, P * tj:P * (tj + 1)],
                        w2_sb[:, 2 * q:2 * q + 2, :],
                        start=(q == 0), stop=False, perf_mode=PM.DoubleRow)
                nc.tensor.matmul(
                    pso, r1l_sb[0:1, P * ti:P * (ti + 1)], b2r_sb[:, :],
                    start=False, stop=True)
                ot = pio.tile([P, D], F32, tag="ot", name=f"ot{ti}")
                nc.vector.scalar_tensor_tensor(
                    out=ot, in0=pso, scalar=1.0 / FSCALE, in1=x2ts[tj],
                    op0=OP.mult, op1=OP.add)
                nc.sync.dma_start(out=out_d[P * ti:P * (ti + 1), :], in_=ot)

            # ---- LN1 tiles for upcoming chunks
            for i in range(4 * c + 6, min(4 * c + 10, NT)):
                ln1_tile(i)
        ctx.close()
    nc.compile()
    return nc


_BUILT = {}


def _get_built():
    if "nc" not in _BUILT:
        plan = make_plan()
        _BUILT["plan"] = plan
        _BUILT["nc"] = build_nc(plan)
    return _BUILT["nc"], _BUILT["plan"]


def kernel(**inputs):
    from concourse.bass_utils import run_bass_kernel_spmd

    nc, plan = _get_built()
    consts = make_consts(inputs, plan)
    x = np.ascontiguousarray(np.asarray(inputs["x"], np.float32))
    in_maps = []
    for b in range(B):
        m = {"x": np.ascontiguousarray(x[b])}
        m.update(consts)
        in_maps.append(m)
    res = run_bass_kernel_spmd(nc, in_maps, core_ids=list(range(B)))
    out = np.stack([res.results[b]["out"] for b in range(B)]).astype(np.float32)
    return out
